# revision 13
# baseline (speedup 1.0000x reference)
"""Causal sparse-attention Bass kernel for Trainium2, 8 NeuronCores.

Reference computation (batch=1, 16 heads, N=2048, D=64):
    dots = (q @ k^T) * D^-0.5 + prev_attn + attn_bias
    dots = where(mask & ~causal_upper, dots, -fmax)
    out  = softmax(dots) @ v

Sharding: 2 heads per core (head-parallel, no cross-core comm).

Per-core algorithm (per head), 16-bit intermediates in fp16:
  - t = prev + bias loaded at exact causal widths (128-granularity): one
    plain cast DMA (prev) + one cast+accumulate DMA (bias) per 128-row strip.
  - S = qT.T @ kT (PE); Sm = S + t (DVE); E = exp(Sm - 3) (ACT);
    Ef = E * m (DVE) with m = mask & causal as 0/1 fp16 (exact zeros).
  - Ef 128x128 blocks are PE-transposed (grouped 4-to-a-PSUM-bank, one ACT
    copy evacuates each group) into E^T class tiles; blocks entirely above
    the causal diagonal are just memset to zero.
  - out'^T[d,i] (+ row-sums in row 64) accumulates on PE over j-chunks with
    v' = [v | 1]; blocks above the diagonal are skipped (block-sparse).
  - degenerate fully-masked rows (reference: uniform softmax over all 2048
    keys) are fixed exactly by a rank-1 K=1 matmul into the same PSUM group:
    rows with sum==0 get numerator += colsum(v'), denominator += 2048.
  - out'^T is PE-transposed back to [i,d], normalized, stored (batched).
"""

import numpy as np
from contextlib import ExitStack

import concourse.bass as bass
import concourse.tile as tile
from concourse import bacc, mybir
from concourse.bass_utils import run_bass_kernel_spmd
from concourse.masks import make_identity

f32 = mybir.dt.float32
f16 = mybir.dt.float16
u8 = mybir.dt.uint8
OP = mybir.AluOpType
AF = mybir.ActivationFunctionType

B, H, N, D = 1, 16, 2048, 64
NCORES = 8
HPC = H // NCORES          # heads per core
NB = N // 128              # 16 i-blocks
EXPSHIFT = -3.0            # keep exp outputs in fp16 range (dots observed < 12)
SCALE = D ** -0.5          # 0.125, exact in fp16

W_CLS = [512, 1024, 1536, 2048]     # rounded band width, class w = ib//4
WT_CLS = [2048, 1536, 1024, 512]    # E^T i-extent for j-block class u = jb//4


def build_module():
    nc = bacc.Bacc("TRN2", target_bir_lowering=False, debug=False)

    q_d = nc.dram_tensor("q", [HPC, N, D], f32, kind="ExternalInput").ap()
    k_d = nc.dram_tensor("k", [HPC, N, D], f32, kind="ExternalInput").ap()
    v_d = nc.dram_tensor("v", [HPC, N, D], f32, kind="ExternalInput").ap()
    prev_d = nc.dram_tensor("prev", [HPC, N, N], f32, kind="ExternalInput").ap()
    bias_d = nc.dram_tensor("bias", [HPC, N, N], f32, kind="ExternalInput").ap()
    mask_d = nc.dram_tensor("mask", [N, N], u8, kind="ExternalInput").ap()
    out_d = nc.dram_tensor("out", [HPC, N, D], f32, kind="ExternalOutput").ap()

    with tile.TileContext(nc) as tc, ExitStack() as ctx:
        singles = ctx.enter_context(tc.tile_pool(name="singles", bufs=1))
        m_pool = ctx.enter_context(tc.tile_pool(name="mp", bufs=1))
        mku_pool = ctx.enter_context(tc.tile_pool(name="mku", bufs=1))
        t_pool = ctx.enter_context(tc.tile_pool(name="t", bufs=3))
        b_pool = ctx.enter_context(tc.tile_pool(name="b", bufs=3))
        qk_pool = ctx.enter_context(tc.tile_pool(name="qk", bufs=2))
        vp_pool = ctx.enter_context(tc.tile_pool(name="vp", bufs=2))
        ef_pool = ctx.enter_context(tc.tile_pool(name="ef", bufs=2))
        sm_pool = ctx.enter_context(tc.tile_pool(name="sm", bufs=2))
        eft_pool = ctx.enter_context(tc.tile_pool(name="eft", bufs=1))
        out_pool = ctx.enter_context(tc.tile_pool(name="outp", bufs=2))
        ps_qk = ctx.enter_context(tc.tile_pool(name="ps_qk", bufs=2, space="PSUM"))
        ps_av = ctx.enter_context(tc.tile_pool(name="ps_av", bufs=2, space="PSUM"))
        ps_xp = ctx.enter_context(tc.tile_pool(name="ps_xp", bufs=3, space="PSUM"))
        ps_cs = ctx.enter_context(tc.tile_pool(name="ps_cs", bufs=1, space="PSUM"))

        # ---------- constants ----------
        ident32 = singles.tile([128, 128], f32)
        make_identity(nc, ident32[:])
        ident16 = singles.tile([128, 128], f16)
        nc.vector.tensor_copy(ident16[:], ident32[:])
        ones_col16 = singles.tile([128, 1], f16)
        nc.gpsimd.memset(ones_col16[:], 1.0)
        expbias = singles.tile([128, 1], f32)
        nc.gpsimd.memset(expbias[:], EXPSHIFT)

        # ---------- mask prep (shared across both heads) ----------
        # m[w]: [128, 4*W] fp16; 1.0 where attended, 0.0 where masked or
        # above the causal diagonal; strips ib=4w+s side by side, each
        # valid only on its exact causal width Wp = 128*(ib+1). The causal
        # boundary only affects the last 128 columns of each strip: multiply
        # them by a lower-triangular 0/1 tile (built once via affine_select).
        tri = singles.tile([128, 128], f16)
        nc.gpsimd.memset(tri[:], 1.0)
        nc.gpsimd.affine_select(
            out=tri[:], in_=tri[:], compare_op=OP.is_ge, fill=0.0,
            base=0, pattern=[[-1, 128]], channel_multiplier=1,
        )
        ms = [None] * 4
        for w in (3, 2, 1, 0):
            W = W_CLS[w]
            m = m_pool.tile([128, 4 * W], f16, tag=f"m{w}", name=f"m{w}")
            for s in range(4):
                ib = 4 * w + s
                Wp = 128 * (ib + 1)
                mku = mku_pool.tile([128, 2048], u8, tag="mku")
                nc.sync.dma_start(
                    mku[:, 0:Wp], mask_d[ib * 128 : (ib + 1) * 128, 0:Wp]
                )
                nc.vector.tensor_scalar(
                    m[:, s * W : s * W + Wp], mku[:, 0:Wp], 0, None,
                    op0=OP.not_equal,
                )
                nc.vector.tensor_tensor(
                    out=m[:, s * W + Wp - 128 : s * W + Wp],
                    in0=m[:, s * W + Wp - 128 : s * W + Wp],
                    in1=tri[:],
                    op=OP.mult,
                )
            ms[w] = m

        # ---------- per-head ----------
        for h in range(HPC):
            # v' = [v | 1] fp16, 16 tiles of [128, 65] packed side by side
            vp = vp_pool.tile([128, NB * 65], f16, tag="vp")
            nc.gpsimd.memset(vp[:], 1.0)
            nc.gpsimd.dma_start(
                vp[:].rearrange("p (t e) -> p t e", e=65)[:, :, 0:D],
                v_d[h].rearrange("(t p) d -> p t d", p=128),
            )

            # colsum of v' -> cs_row [1, 65] fp16 (used as K=1 lhsT)
            cs_ps = ps_cs.tile([1, 65], f32, tag="ps_cs")
            for jc in range(NB):
                nc.tensor.matmul(
                    cs_ps[:],
                    ones_col16[:],
                    vp[:, jc * 65 : (jc + 1) * 65],
                    start=(jc == 0),
                    stop=(jc == NB - 1),
                )
            cs_row = singles.tile([1, 65], f16, tag=f"cs_row{h}")
            nc.vector.tensor_copy(cs_row[:], cs_ps[:])

            # qT (scaled) and kT: cast-load fp16, PE-transpose 128x64 tiles;
            # 4 transposes share one PSUM tile, evacuated in one ACT copy.
            q_st = qk_pool.tile([128, NB * D], f16, tag="q_st")
            nc.gpsimd.dma_start(
                q_st[:].rearrange("p (t d) -> p t d", d=D),
                q_d[h].rearrange("(t p) d -> p t d", p=128),
            )
            k_st = qk_pool.tile([128, NB * D], f16, tag="k_st")
            nc.gpsimd.dma_start(
                k_st[:].rearrange("p (t d) -> p t d", d=D),
                k_d[h].rearrange("(t p) d -> p t d", p=128),
            )
            qT = qk_pool.tile([128, N], f16, tag="qT")
            kT = qk_pool.tile([128, N], f16, tag="kT")
            for st, dst, scl in ((q_st, qT, SCALE), (k_st, kT, 1.0)):
                for g in range(4):
                    ptr = ps_xp.tile([128, 512], f16, tag="ps_xp")
                    for r in range(4):
                        ib = 4 * g + r
                        nc.tensor.transpose(
                            ptr[0:64, r * 128 : (r + 1) * 128],
                            st[:, ib * D : (ib + 1) * D],
                            ident16[:],
                        )
                    if scl == 1.0:
                        nc.scalar.copy(
                            dst[0:64, g * 512 : (g + 1) * 512], ptr[0:64, :]
                        )
                    else:
                        nc.scalar.mul(
                            dst[0:64, g * 512 : (g + 1) * 512], ptr[0:64, :], scl
                        )

            # E^T class tiles: u = jb//4, i-extent WT = 2048 - 512*u; block
            # jb = 4u+r lives at [:, r*WT + (i - 512*u)] for i in [512u, 2048).
            # The leading 128*r columns of block r (i < 128*jb, above the
            # diagonal) are never produced -> memset zero once per head.
            efT = []
            for u in range(4):
                eft = eft_pool.tile(
                    [128, 4 * WT_CLS[u]], f16, tag=f"efT{u}", name=f"efT{u}_{h}"
                )
                for r in range(1, 4):
                    nc.gpsimd.memset(
                        eft[:, r * WT_CLS[u] : r * WT_CLS[u] + 128 * r], 0.0
                    )
                efT.append(eft)

            for w in (3, 2, 1, 0):
                W = W_CLS[w]
                # t = prev + bias at exact causal widths (plain + accum DMA)
                t_w = t_pool.tile([128, 4 * W], f16, tag="t")
                for s in range(4):
                    ib = 4 * w + s
                    Wp = 128 * (ib + 1)
                    nc.gpsimd.dma_start(
                        t_w[:, s * W : s * W + Wp],
                        prev_d[h][ib * 128 : (ib + 1) * 128, 0:Wp],
                    )
                    b_s = b_pool.tile([128, 2048], f16, tag="b")
                    nc.gpsimd.dma_start(
                        b_s[:, 0:Wp],
                        bias_d[h][ib * 128 : (ib + 1) * 128, 0:Wp],
                    )
                    nc.vector.tensor_tensor(
                        out=t_w[:, s * W : s * W + Wp],
                        in0=t_w[:, s * W : s * W + Wp],
                        in1=b_s[:, 0:Wp],
                        op=OP.add,
                    )

                for s in range(4):
                    ib = 4 * w + s
                    Wp = 128 * (ib + 1)
                    sm = sm_pool.tile([128, W], f16, tag="sm")
                    for jc5 in range(w + 1):
                        cw = 512 if jc5 < w else 128 * (s + 1)
                        ps = ps_qk.tile([128, 512], f32, tag="ps_qk")
                        nc.tensor.matmul(
                            ps[:, 0:cw],
                            qT[0:64, ib * 128 : (ib + 1) * 128],
                            kT[0:64, jc5 * 512 : jc5 * 512 + cw],
                            start=True,
                            stop=True,
                        )
                        nc.vector.tensor_tensor(
                            out=sm[:, jc5 * 512 : jc5 * 512 + cw],
                            in0=ps[:, 0:cw],
                            in1=t_w[:, s * W + jc5 * 512 : s * W + jc5 * 512 + cw],
                            op=OP.add,
                        )
                    ef = ef_pool.tile([128, W], f16, tag="ef")
                    nc.scalar.activation(
                        ef[:, 0:Wp], sm[:, 0:Wp], AF.Exp, bias=expbias[:], scale=1.0
                    )
                    nc.vector.tensor_tensor(
                        out=ef[:, 0:Wp],
                        in0=ef[:, 0:Wp],
                        in1=ms[w][:, s * W : s * W + Wp],
                        op=OP.mult,
                    )
                    # transpose Ef 128x128 blocks, grouped per 512-chunk;
                    # class-3 full chunks ride the DMA xbar (SP is idle),
                    # the rest are PE transposes + one ACT evac per group
                    for u in range(w + 1):
                        nr = 4 if u < w else s + 1
                        WT = WT_CLS[u]
                        dst3 = efT[u][:].rearrange("p (r i) -> p r i", i=WT)[
                            :, 0:nr, ib * 128 - 512 * u : ib * 128 - 512 * u + 128
                        ]
                        xp = ps_xp.tile([128, 512], f16, tag="ps_xp")
                        for r in range(nr):
                            nc.tensor.transpose(
                                xp[:, r * 128 : (r + 1) * 128],
                                ef[:, u * 512 + r * 128 : u * 512 + (r + 1) * 128],
                                ident16[:],
                            )
                        nc.scalar.copy(
                            dst3, xp[:, 0 : nr * 128].rearrange(
                                "p (r c) -> p r c", c=128
                            ),
                        )

                # ---- AV for i-chunk c == w (strips 4w..4w+3 transposed) ----
                c = w
                po = ps_av.tile([65, 512], f32, tag="ps_av")
                for jb in range(4 * c + 4):
                    u, r = jb // 4, jb % 4
                    off = r * WT_CLS[u] + (c * 512 - 512 * u)
                    nc.tensor.matmul(
                        po[:],
                        vp[:, jb * 65 : (jb + 1) * 65],
                        efT[u][:, off : off + 512],
                        start=(jb == 0),
                        stop=False,
                    )
                # rank-1 degenerate-row fix: rows with sum==0 get
                # numerator += colsum(v'), denominator += 2048
                dr = out_pool.tile([1, 512], f16, tag="dr")
                nc.vector.tensor_scalar(
                    dr[:], po[64:65, :], 0.0, None, op0=OP.is_equal
                )
                nc.tensor.matmul(
                    po[:], cs_row[:], dr[:], start=False, stop=True,
                    skip_group_check=True,
                )
                o_sb = out_pool.tile([65, 512], f32, tag="o_sb")
                nc.scalar.copy(o_sb[:], po[:])
                # transpose back to [i, d] (4 PE transposes share one bank)
                pt = ps_cs.tile([128, 4 * 65], f32, tag="ps_cs")
                for r in range(4):
                    nc.tensor.transpose(
                        pt[:, r * 65 : (r + 1) * 65],
                        o_sb[:, r * 128 : (r + 1) * 128],
                        ident32[0:65, 0:65],
                    )
                ot = out_pool.tile([128, 4 * 65], f32, tag="ot")
                nc.vector.tensor_copy(ot[:], pt[:])
                of = out_pool.tile([128, 4 * D], f32, tag="of")
                for r in range(4):
                    rcp = out_pool.tile([128, 1], f32, tag="rcp")
                    nc.vector.reciprocal(rcp[:], ot[:, r * 65 + 64 : r * 65 + 65])
                    nc.vector.tensor_scalar(
                        of[:, r * D : (r + 1) * D],
                        ot[:, r * 65 : r * 65 + 64],
                        rcp[:],
                        None,
                        op0=OP.mult,
                    )
                nc.sync.dma_start(
                    out_d[h, 512 * c : 512 * (c + 1), :].rearrange(
                        "(r p) d -> p r d", p=128
                    ),
                    of[:].rearrange("p (r d) -> p r d", d=D),
                )

    nc.compile()
    return nc


_NC_CACHE = None


def _get_nc():
    global _NC_CACHE
    if _NC_CACHE is None:
        _NC_CACHE = build_module()
    return _NC_CACHE


def make_in_maps(q, k, v, mask, attn_bias, prev_attn):
    q = np.asarray(q, dtype=np.float32).reshape(H, N, D)
    k = np.asarray(k, dtype=np.float32).reshape(H, N, D)
    v = np.asarray(v, dtype=np.float32).reshape(H, N, D)
    prev = np.asarray(prev_attn, dtype=np.float32).reshape(H, N, N)
    bias = np.asarray(attn_bias, dtype=np.float32).reshape(H, N, N)
    mku = np.asarray(mask).reshape(N, N).astype(np.uint8)
    in_maps = []
    for c in range(NCORES):
        sl = slice(c * HPC, (c + 1) * HPC)
        in_maps.append(
            {
                "q": np.ascontiguousarray(q[sl]),
                "k": np.ascontiguousarray(k[sl]),
                "v": np.ascontiguousarray(v[sl]),
                "prev": np.ascontiguousarray(prev[sl]),
                "bias": np.ascontiguousarray(bias[sl]),
                "mask": mku,
            }
        )
    return in_maps


def run_kernel(q, k, v, mask, attn_bias, prev_attn, trace=False, **trace_kw):
    nc = _get_nc()
    in_maps = make_in_maps(q, k, v, mask, attn_bias, prev_attn)
    res = run_bass_kernel_spmd(
        nc, in_maps, list(range(NCORES)), trace=trace, **trace_kw
    )
    outs = [res.results[c]["out"] for c in range(NCORES)]
    full = np.concatenate(outs, axis=0).reshape(B, H, N, D).astype(np.float32)
    return full, res


def kernel(q, k, v, mask, attn_bias, prev_attn):
    out, _ = run_kernel(q, k, v, mask, attn_bias, prev_attn, trace=False)
    return out


# revision 14
# speedup vs baseline: 1.2430x; 1.2430x over previous
"""Causal sparse-attention Bass kernel for Trainium2, 8 NeuronCores.

Reference computation (batch=1, 16 heads, N=2048, D=64):
    dots = (q @ k^T) * D^-0.5 + prev_attn + attn_bias
    dots = where(mask & ~causal_upper, dots, -fmax)
    out  = softmax(dots) @ v

Sharding: 2 heads per core (head-parallel, no cross-core comm).

Per-core algorithm (per head), 16-bit intermediates in fp16:
  - t = prev + bias loaded at exact causal widths (128-granularity): one
    plain cast DMA (prev) + one cast+accumulate DMA (bias) per 128-row strip.
  - S = qT.T @ kT (PE); Sm = S + t (DVE); E = exp(Sm - 3) (ACT);
    Ef = E * m (DVE) with m = mask & causal as 0/1 fp16 (exact zeros).
  - Ef 128x128 blocks are PE-transposed (grouped 4-to-a-PSUM-bank, one ACT
    copy evacuates each group) into E^T class tiles; blocks entirely above
    the causal diagonal are just memset to zero.
  - out'^T[d,i] (+ row-sums in row 64) accumulates on PE over j-chunks with
    v' = [v | 1]; blocks above the diagonal are skipped (block-sparse).
  - degenerate fully-masked rows (reference: uniform softmax over all 2048
    keys) are fixed exactly by a rank-1 K=1 matmul into the same PSUM group:
    rows with sum==0 get numerator += colsum(v'), denominator += 2048.
  - out'^T is PE-transposed back to [i,d], normalized, stored (batched).
"""

import numpy as np
from contextlib import ExitStack

import concourse.bass as bass
import concourse.tile as tile
from concourse import bacc, mybir
from concourse.bass_utils import run_bass_kernel_spmd
from concourse.masks import make_identity

f32 = mybir.dt.float32
f16 = mybir.dt.float16
u8 = mybir.dt.uint8
OP = mybir.AluOpType
AF = mybir.ActivationFunctionType

B, H, N, D = 1, 16, 2048, 64
NCORES = 8
HPC = H // NCORES          # heads per core
NB = N // 128              # 16 i-blocks
EXPSHIFT = -3.0            # keep exp outputs in fp16 range (dots observed < 12)
SCALE = D ** -0.5          # 0.125, exact in fp16

W_CLS = [512, 1024, 1536, 2048]     # rounded band width, class w = ib//4
WT_CLS = [2048, 1536, 1024, 512]    # E^T i-extent for j-block class u = jb//4


def build_module():
    nc = bacc.Bacc("TRN2", target_bir_lowering=False, debug=False)

    q_d = nc.dram_tensor("q", [HPC, N, D], f32, kind="ExternalInput").ap()
    k_d = nc.dram_tensor("k", [HPC, N, D], f32, kind="ExternalInput").ap()
    v_d = nc.dram_tensor("v", [HPC, N, D], f32, kind="ExternalInput").ap()
    prev_d = nc.dram_tensor("prev", [HPC, N, N], f32, kind="ExternalInput").ap()
    bias_d = nc.dram_tensor("bias", [HPC, N, N], f32, kind="ExternalInput").ap()
    mask_d = nc.dram_tensor("mask", [N, N], u8, kind="ExternalInput").ap()
    out_d = nc.dram_tensor("out", [HPC, N, D], f32, kind="ExternalOutput").ap()

    with tile.TileContext(nc) as tc, ExitStack() as ctx:
        singles = ctx.enter_context(tc.tile_pool(name="singles", bufs=1))
        m_pool = ctx.enter_context(tc.tile_pool(name="mp", bufs=1))
        mku_pool = ctx.enter_context(tc.tile_pool(name="mku", bufs=1))
        t_pool = ctx.enter_context(tc.tile_pool(name="t", bufs=3))
        b_pool = ctx.enter_context(tc.tile_pool(name="b", bufs=3))
        qk_pool = ctx.enter_context(tc.tile_pool(name="qk", bufs=2))
        vp_pool = ctx.enter_context(tc.tile_pool(name="vp", bufs=2))
        ef_pool = ctx.enter_context(tc.tile_pool(name="ef", bufs=2))
        sm_pool = ctx.enter_context(tc.tile_pool(name="sm", bufs=2))
        eft_pool = ctx.enter_context(tc.tile_pool(name="eft", bufs=1))
        out_pool = ctx.enter_context(tc.tile_pool(name="outp", bufs=2))
        ps_qk = ctx.enter_context(tc.tile_pool(name="ps_qk", bufs=2, space="PSUM"))
        ps_av = ctx.enter_context(tc.tile_pool(name="ps_av", bufs=2, space="PSUM"))
        ps_xp = ctx.enter_context(tc.tile_pool(name="ps_xp", bufs=2, space="PSUM"))
        ps_pt = ctx.enter_context(tc.tile_pool(name="ps_pt", bufs=1, space="PSUM"))
        ps_cs = ctx.enter_context(tc.tile_pool(name="ps_cs", bufs=1, space="PSUM"))

        # ---------- constants ----------
        ident32 = singles.tile([128, 128], f32)
        make_identity(nc, ident32[:])
        ident16 = singles.tile([128, 128], f16)
        nc.vector.tensor_copy(ident16[:], ident32[:])
        ones_col16 = singles.tile([128, 1], f16)
        nc.gpsimd.memset(ones_col16[:], 1.0)
        expbias = singles.tile([128, 1], f32)
        nc.gpsimd.memset(expbias[:], EXPSHIFT)

        # ---------- mask prep (shared across both heads) ----------
        # m[w]: [128, 4*W] fp16; 1.0 where attended, 0.0 where masked or
        # above the causal diagonal; strips ib=4w+s side by side, each
        # valid only on its exact causal width Wp = 128*(ib+1). The causal
        # boundary only affects the last 128 columns of each strip: multiply
        # them by a lower-triangular 0/1 tile (built once via affine_select).
        tri = singles.tile([128, 128], f16)
        nc.gpsimd.memset(tri[:], 1.0)
        nc.gpsimd.affine_select(
            out=tri[:], in_=tri[:], compare_op=OP.is_ge, fill=0.0,
            base=0, pattern=[[-1, 128]], channel_multiplier=1,
        )
        ms = [None] * 4
        for w in (3, 2, 1, 0):
            W = W_CLS[w]
            m = m_pool.tile([128, 4 * W], f16, tag=f"m{w}", name=f"m{w}")
            for s in range(4):
                ib = 4 * w + s
                Wp = 128 * (ib + 1)
                mku = mku_pool.tile([128, 2048], u8, tag="mku")
                nc.sync.dma_start(
                    mku[:, 0:Wp], mask_d[ib * 128 : (ib + 1) * 128, 0:Wp]
                )
                nc.vector.tensor_scalar(
                    m[:, s * W : s * W + Wp], mku[:, 0:Wp], 0, None,
                    op0=OP.not_equal,
                )
                nc.vector.tensor_tensor(
                    out=m[:, s * W + Wp - 128 : s * W + Wp],
                    in0=m[:, s * W + Wp - 128 : s * W + Wp],
                    in1=tri[:],
                    op=OP.mult,
                )
            ms[w] = m

        # ---------- per-head ----------
        for h in range(HPC):
            # v' = [v | 1] fp16, 16 tiles of [128, 65] packed side by side
            vp = vp_pool.tile([128, NB * 65], f16, tag="vp")
            nc.gpsimd.memset(vp[:], 1.0)
            nc.gpsimd.dma_start(
                vp[:].rearrange("p (t e) -> p t e", e=65)[:, :, 0:D],
                v_d[h].rearrange("(t p) d -> p t d", p=128),
            )

            # colsum of v' -> cs_row [1, 65] fp16 (used as K=1 lhsT)
            cs_ps = ps_cs.tile([1, 65], f32, tag="ps_cs")
            for jc in range(NB):
                nc.tensor.matmul(
                    cs_ps[:],
                    ones_col16[:],
                    vp[:, jc * 65 : (jc + 1) * 65],
                    start=(jc == 0),
                    stop=(jc == NB - 1),
                )
            cs_row = singles.tile([1, 65], f16, tag=f"cs_row{h}")
            nc.vector.tensor_copy(cs_row[:], cs_ps[:])

            # qT (scaled) and kT: cast-load fp16, PE-transpose 128x64 tiles;
            # 4 transposes share one PSUM tile, evacuated in one ACT copy.
            q_st = qk_pool.tile([128, NB * D], f16, tag="q_st")
            nc.gpsimd.dma_start(
                q_st[:].rearrange("p (t d) -> p t d", d=D),
                q_d[h].rearrange("(t p) d -> p t d", p=128),
            )
            k_st = qk_pool.tile([128, NB * D], f16, tag="k_st")
            nc.gpsimd.dma_start(
                k_st[:].rearrange("p (t d) -> p t d", d=D),
                k_d[h].rearrange("(t p) d -> p t d", p=128),
            )
            qT = qk_pool.tile([128, N], f16, tag="qT")
            kT = qk_pool.tile([128, N], f16, tag="kT")
            for st, dst, scl in ((q_st, qT, SCALE), (k_st, kT, 1.0)):
                for g in range(4):
                    ptr = ps_xp.tile([128, 512], f16, tag="ps_xp")
                    for r in range(4):
                        ib = 4 * g + r
                        nc.tensor.transpose(
                            ptr[0:64, r * 128 : (r + 1) * 128],
                            st[:, ib * D : (ib + 1) * D],
                            ident16[:],
                        )
                    if scl == 1.0:
                        nc.scalar.copy(
                            dst[0:64, g * 512 : (g + 1) * 512], ptr[0:64, :]
                        )
                    else:
                        nc.scalar.mul(
                            dst[0:64, g * 512 : (g + 1) * 512], ptr[0:64, :], scl
                        )

            # E^T class tiles: u = jb//4, i-extent WT = 2048 - 512*u; block
            # jb = 4u+r lives at [:, r*WT + (i - 512*u)] for i in [512u, 2048).
            # The leading 128*r columns of block r (i < 128*jb, above the
            # diagonal) are never produced -> memset zero once per head.
            efT = []
            for u in range(4):
                eft = eft_pool.tile(
                    [128, 4 * WT_CLS[u]], f16, tag=f"efT{u}", name=f"efT{u}_{h}"
                )
                for r in range(1, 4):
                    nc.gpsimd.memset(
                        eft[:, r * WT_CLS[u] : r * WT_CLS[u] + 128 * r], 0.0
                    )
                efT.append(eft)

            for w in (3, 2, 1, 0):
                W = W_CLS[w]
                # t = prev + bias at exact causal widths (plain + accum DMA)
                t_w = t_pool.tile([128, 4 * W], f16, tag="t")
                for s in range(4):
                    ib = 4 * w + s
                    Wp = 128 * (ib + 1)
                    nc.gpsimd.dma_start(
                        t_w[:, s * W : s * W + Wp],
                        prev_d[h][ib * 128 : (ib + 1) * 128, 0:Wp],
                    )
                    b_s = b_pool.tile([128, 2048], f16, tag="b")
                    nc.gpsimd.dma_start(
                        b_s[:, 0:Wp],
                        bias_d[h][ib * 128 : (ib + 1) * 128, 0:Wp],
                    )
                    nc.vector.tensor_tensor(
                        out=t_w[:, s * W : s * W + Wp],
                        in0=t_w[:, s * W : s * W + Wp],
                        in1=b_s[:, 0:Wp],
                        op=OP.add,
                    )

                for s in range(4):
                    ib = 4 * w + s
                    Wp = 128 * (ib + 1)
                    sm = sm_pool.tile([128, W], f16, tag="sm")
                    for jc5 in range(w + 1):
                        cw = 512 if jc5 < w else 128 * (s + 1)
                        ps = ps_qk.tile([128, 512], f32, tag="ps_qk")
                        nc.tensor.matmul(
                            ps[:, 0:cw],
                            qT[0:64, ib * 128 : (ib + 1) * 128],
                            kT[0:64, jc5 * 512 : jc5 * 512 + cw],
                            start=True,
                            stop=True,
                        )
                        nc.vector.tensor_tensor(
                            out=sm[:, jc5 * 512 : jc5 * 512 + cw],
                            in0=ps[:, 0:cw],
                            in1=t_w[:, s * W + jc5 * 512 : s * W + jc5 * 512 + cw],
                            op=OP.add,
                        )
                    ef = ef_pool.tile([128, W], f16, tag="ef")
                    nc.scalar.activation(
                        ef[:, 0:Wp], sm[:, 0:Wp], AF.Exp, bias=expbias[:], scale=1.0
                    )
                    nc.vector.tensor_tensor(
                        out=ef[:, 0:Wp],
                        in0=ef[:, 0:Wp],
                        in1=ms[w][:, s * W : s * W + Wp],
                        op=OP.mult,
                    )
                    # transpose Ef 128x128 blocks, grouped per 512-chunk;
                    # class-3 full chunks ride the DMA xbar (SP is idle),
                    # the rest are PE transposes + one ACT evac per group
                    for u in range(w + 1):
                        nr = 4 if u < w else s + 1
                        WT = WT_CLS[u]
                        dst3 = efT[u][:].rearrange("p (r i) -> p r i", i=WT)[
                            :, 0:nr, ib * 128 - 512 * u : ib * 128 - 512 * u + 128
                        ]
                        xp = ps_xp.tile([128, 512], f16, tag="ps_xp")
                        for r in range(nr):
                            nc.tensor.transpose(
                                xp[:, r * 128 : (r + 1) * 128],
                                ef[:, u * 512 + r * 128 : u * 512 + (r + 1) * 128],
                                ident16[:],
                            )
                        nc.scalar.copy(
                            dst3, xp[:, 0 : nr * 128].rearrange(
                                "p (r c) -> p r c", c=128
                            ),
                        )

                # ---- AV for i-chunk c == w (strips 4w..4w+3 transposed) ----
                c = w
                po = ps_av.tile([65, 512], f32, tag="ps_av")
                for jb in range(4 * c + 4):
                    u, r = jb // 4, jb % 4
                    off = r * WT_CLS[u] + (c * 512 - 512 * u)
                    nc.tensor.matmul(
                        po[:],
                        vp[:, jb * 65 : (jb + 1) * 65],
                        efT[u][:, off : off + 512],
                        start=(jb == 0),
                        stop=False,
                    )
                # rank-1 degenerate-row fix: rows with sum==0 get
                # numerator += colsum(v'), denominator += 2048
                dr = out_pool.tile([1, 512], f16, tag="dr")
                nc.vector.tensor_scalar(
                    dr[:], po[64:65, :], 0.0, None, op0=OP.is_equal
                )
                nc.tensor.matmul(
                    po[:], cs_row[:], dr[:], start=False, stop=True,
                    skip_group_check=True,
                )
                o_sb = out_pool.tile([65, 512], f32, tag="o_sb")
                nc.scalar.copy(o_sb[:], po[:])
                # transpose back to [i, d] (4 PE transposes share one bank)
                pt = ps_pt.tile([128, 4 * 65], f32, tag="ps_pt")
                for r in range(4):
                    nc.tensor.transpose(
                        pt[:, r * 65 : (r + 1) * 65],
                        o_sb[:, r * 128 : (r + 1) * 128],
                        ident32[0:65, 0:65],
                    )
                ot = out_pool.tile([128, 4 * 65], f32, tag="ot")
                nc.vector.tensor_copy(ot[:], pt[:])
                of = out_pool.tile([128, 4 * D], f32, tag="of")
                for r in range(4):
                    rcp = out_pool.tile([128, 1], f32, tag="rcp")
                    nc.vector.reciprocal(rcp[:], ot[:, r * 65 + 64 : r * 65 + 65])
                    nc.vector.tensor_scalar(
                        of[:, r * D : (r + 1) * D],
                        ot[:, r * 65 : r * 65 + 64],
                        rcp[:],
                        None,
                        op0=OP.mult,
                    )
                nc.sync.dma_start(
                    out_d[h, 512 * c : 512 * (c + 1), :].rearrange(
                        "(r p) d -> p r d", p=128
                    ),
                    of[:].rearrange("p (r d) -> p r d", d=D),
                )

    nc.compile()
    return nc


_NC_CACHE = None


def _get_nc():
    global _NC_CACHE
    if _NC_CACHE is None:
        _NC_CACHE = build_module()
    return _NC_CACHE


def make_in_maps(q, k, v, mask, attn_bias, prev_attn):
    q = np.asarray(q, dtype=np.float32).reshape(H, N, D)
    k = np.asarray(k, dtype=np.float32).reshape(H, N, D)
    v = np.asarray(v, dtype=np.float32).reshape(H, N, D)
    prev = np.asarray(prev_attn, dtype=np.float32).reshape(H, N, N)
    bias = np.asarray(attn_bias, dtype=np.float32).reshape(H, N, N)
    mku = np.asarray(mask).reshape(N, N).astype(np.uint8)
    in_maps = []
    for c in range(NCORES):
        sl = slice(c * HPC, (c + 1) * HPC)
        in_maps.append(
            {
                "q": np.ascontiguousarray(q[sl]),
                "k": np.ascontiguousarray(k[sl]),
                "v": np.ascontiguousarray(v[sl]),
                "prev": np.ascontiguousarray(prev[sl]),
                "bias": np.ascontiguousarray(bias[sl]),
                "mask": mku,
            }
        )
    return in_maps


def run_kernel(q, k, v, mask, attn_bias, prev_attn, trace=False, **trace_kw):
    nc = _get_nc()
    in_maps = make_in_maps(q, k, v, mask, attn_bias, prev_attn)
    res = run_bass_kernel_spmd(
        nc, in_maps, list(range(NCORES)), trace=trace, **trace_kw
    )
    outs = [res.results[c]["out"] for c in range(NCORES)]
    full = np.concatenate(outs, axis=0).reshape(B, H, N, D).astype(np.float32)
    return full, res


def kernel(q, k, v, mask, attn_bias, prev_attn):
    out, _ = run_kernel(q, k, v, mask, attn_bias, prev_attn, trace=False)
    return out
